# revision 175
# baseline (speedup 1.0000x reference)
"""Trainium2 Bass kernel for an attention block (RMSNorm + fused QKV + RoPE +
causal MHA + output projection), Megatron-style head sharding over 8 NeuronCores.

Shapes (hardcoded): B=2, T=2048, C=1024, H=16, D=64. Each core handles 2 heads.

Final design (235.5us baseline -> 164.0us):
- engine balance: PE matmuls / DVE elementwise+PSUM evac / ACT exp+copies /
  GPSIMD squares+partition-broadcasts / two HWDGE DMA queues (SP=x-stream,
  ACT=constants)
- rsqrt via ACT ln -> exp(-0.5x - 6ln2) with a manually preloaded table that
  covers both Ln and Exp (zero activation-table reloads); the -6ln2 bias
  undoes the x64 weight upscaling used for fp8
- V-projection and the sum-of-squares reduction run in fp8e4m3 DoubleRow
  (2 contraction rows/partition, half PE cost); Q/K stay bf16 because score
  noise through the softmax is the dominant error path; V of the first 512
  tokens per batch stays bf16 since early tokens get no softmax averaging
- both heads' scores packed in one [128,1024] 2-bank PSUM tile, single fused
  exp per k-tile with scale=1/8 (the 1/sqrt(D) fold); causal masking is a
  -240 additive bias matmul into PSUM before the exp (off the critical path)
- score matmuls emitted one k-tile ahead of AV so PE never waits on exp
- softmax denominator from an augmented [v | 1] AV matmul row; normalize via
  DVE reciprocal + GPSIMD partition-broadcast (PE outer product on the tail)
- emission interleaving: batch-1 prologue and previous-chunk o_proj quanta
  are pumped between attention k-tiles to fill the in-order PE stream; PSUM
  lives in a disciplined 2-slot ring (readers always emitted before reuse)

Host: shards weights (x64 into fp8/bf16 layouts incl. the dual-row V blocks),
precomputes RoPE tables / causal bias / identities, sums the 8 partial
outputs in fp32, adds b_o. b_qkv supported only as zeros (spec fill=zeros).
"""

import numpy as np
import ml_dtypes

B, T, C, H, D = 2, 2048, 1024, 16, 64
BT = B * T
NCORES = 8
HPC = H // NCORES               # heads per core = 2
CSH = HPC * D                   # per-core attention channels = 128
EPS = 1e-5
ROPE_BASE = 10000.0

CT = C // 128                   # 8 c-tiles
BTC = BT // 512                 # 8 bt chunks of 512
QC = T // 512                   # 4 q chunks of 512 per batch
VSTR = 80                       # per-ktile stride in v_aug (32B aligned)

BF16 = ml_dtypes.bfloat16

_cache = {}


def _host_tables():
    half = D // 2
    inv_freq = 1.0 / (ROPE_BASE ** (np.arange(half, dtype=np.float64) / half))
    t = np.arange(T, dtype=np.float64)
    ang = t[None, :] * inv_freq[:, None]
    ang = np.concatenate([ang, ang], axis=0)      # [64, T]
    cos = np.cos(ang)
    sin = np.sin(ang)
    sgn = np.where(np.arange(D) < half, -1.0, 1.0)[:, None]
    sinS = sin * sgn
    cosT = np.tile(cos, (2, 1)).astype(BF16)      # [128, T]
    sinT = np.tile(sinS, (2, 1)).astype(BF16)
    # additive causal bias for the diagonal 128x128 score blocks; the exp
    # applies scale=1/8 (the 1/sqrt(D) fold), so -240 lands at exp(-30)~=0
    tri = np.where(np.arange(128)[:, None] <= np.arange(128)[None, :],
                   0.0, -240.0).astype(BF16)
    eye = np.concatenate([np.eye(D), np.eye(D)], axis=0).astype(BF16)  # [128,D]
    eye128 = np.eye(128, dtype=BF16)
    sh = np.r_[np.arange(32, 64), np.arange(0, 32),
               np.arange(96, 128), np.arange(64, 96)]
    perm = np.zeros((128, 128), dtype=BF16)
    perm[sh, np.arange(128)] = 1.0    # lhsT[s, p] = 1 iff s = sh(p)
    return cosT, sinT, tri, eye, eye128, perm


def _build():
    import concourse.bacc as bacc
    import concourse.mybir as mybir
    from concourse.tile import TileContext
    from contextlib import ExitStack

    f32 = mybir.dt.float32
    bf16 = mybir.dt.bfloat16
    fp8 = mybir.dt.float8e4
    DROW = mybir.MatmulPerfMode.DoubleRow
    MUL = mybir.AluOpType.mult
    ADD = mybir.AluOpType.add
    EXP = mybir.ActivationFunctionType.Exp
    LN = mybir.ActivationFunctionType.Ln

    nc = bacc.Bacc("TRN2", target_bir_lowering=False, debug=False,
                   num_devices=NCORES)

    xT_in = nc.dram_tensor("xT", [C, BT], bf16, kind="ExternalInput").ap()
    x8_in = nc.dram_tensor("x8", [C, BT], fp8, kind="ExternalInput").ap()
    wT_in = nc.dram_tensor("wT", [C, 2 * CSH], bf16, kind="ExternalInput").ap()
    # w8v layout: [p, (pair P, i, m)] — each P slice is a contiguous [2, 128]
    # dual-row block as the fp8 Ldweights ISA requires
    w8v_in = nc.dram_tensor("w8v", [128, (CT // 2) * 2 * CSH], fp8,
                            kind="ExternalInput").ap()
    wTv_in = nc.dram_tensor("wTv", [C, CSH], bf16, kind="ExternalInput").ap()
    woT_in = nc.dram_tensor("woT", [CSH, C], bf16, kind="ExternalInput").ap()
    cos_in = nc.dram_tensor("cosT", [128, T], bf16, kind="ExternalInput").ap()
    sin_in = nc.dram_tensor("sinT", [128, T], bf16, kind="ExternalInput").ap()
    tri_in = nc.dram_tensor("tri", [128, 128], bf16, kind="ExternalInput").ap()
    eye_in = nc.dram_tensor("eye", [128, D], bf16, kind="ExternalInput").ap()
    eye128_in = nc.dram_tensor("eye128", [128, 128], bf16,
                               kind="ExternalInput").ap()
    perm_in = nc.dram_tensor("perm", [128, 128], bf16, kind="ExternalInput").ap()
    out_dram = nc.dram_tensor("out", [BT, C], bf16, kind="ExternalOutput").ap()

    with nc.allow_low_precision(reason="fp32r broadcast operands are exact for 1.0*x"), \
         TileContext(nc) as tc, ExitStack() as outer:
        cpool = outer.enter_context(tc.tile_pool(name="consts", bufs=1))
        work = outer.enter_context(tc.tile_pool(name="work", bufs=3))

        # first x chunk DMA goes out before the big constant loads so the
        # pipeline starts immediately
        xtc0 = work.tile([128, CT * 512], bf16, tag="xtc", name="xtc0", bufs=5)
        x8tc0 = work.tile([128, CT * 512], fp8, tag="x8", name="x8tc0", bufs=5)
        x0r = xtc0[:].rearrange("p (ct f) -> p ct f", f=512)
        xi0 = xT_in[:, 0:512].rearrange("(ct p) f -> p ct f", p=128)
        nc.sync.dma_start(out=x0r[:, 0:4], in_=xi0[:, 0:4])
        nc.sync.dma_start(out=x0r[:, 4:8], in_=xi0[:, 4:8])

        w_sb = cpool.tile([128, CT * 2 * CSH], bf16)
        w8v_sb = cpool.tile([128, (CT // 2) * 2 * CSH], fp8)
        wv_sb = cpool.tile([128, CT * CSH], bf16)
        woT_sb = cpool.tile([128, C], bf16)
        tri_sb = cpool.tile([128, 128], bf16)
        eye_sb = cpool.tile([128, D], bf16)
        eye128_sb = cpool.tile([128, 128], bf16)
        perm_sb = cpool.tile([128, 128], bf16)
        onesb8 = cpool.tile([128, 32], fp8)
        onesb_bf = cpool.tile([128, 1], bf16)
        ones1_f32 = cpool.tile([1, 128], f32)
        ones64_bf = cpool.tile([1, 64], bf16)
        eps_sb = cpool.tile([1, 1], f32)
        m6ln2_sb = cpool.tile([1, 1], f32)
        cos_sb = cpool.tile([128, T], bf16)
        sin_sb = cpool.tile([128, T], bf16)
        nc.vector.memset(onesb8[:], 1.0)
        nc.vector.memset(onesb_bf[:], 1.0)
        nc.vector.memset(ones1_f32[:], 1.0)
        nc.vector.memset(ones64_bf[:], 1.0)
        nc.vector.memset(eps_sb[:], EPS)
        nc.vector.memset(m6ln2_sb[:], -6.0 * float(np.log(2.0)))
        # preload the one activation table that covers both Ln and Exp so the
        # compiler's table-load pass never inserts a reload
        nc.scalar.add_instruction(mybir.InstLoadActFuncSet(
            name=nc.get_next_instruction_name(), ins=[], outs=[],
            act_func_set_id=6))
        # SP HWDGE queue carries only the x-chunk stream (w first, it gates
        # the first QKV); everything else rides the idle ACT HWDGE queue
        nc.sync.dma_start(
            out=w_sb[:].rearrange("p (ct f) -> p ct f", f=2 * CSH),
            in_=wT_in[:].rearrange("(ct p) f -> p ct f", p=128))
        nc.sync.dma_start(
            out=x8tc0[:].rearrange("p (ct f) -> p ct f", f=512),
            in_=x8_in[:, 0:512].rearrange("(ct p) f -> p ct f", p=128))
        nc.sync.dma_start(out=w8v_sb[:], in_=w8v_in[:])
        nc.scalar.dma_start(out=cos_sb[:], in_=cos_in[:])
        nc.scalar.dma_start(out=sin_sb[:], in_=sin_in[:])
        nc.scalar.dma_start(out=perm_sb[:], in_=perm_in[:])
        nc.scalar.dma_start(out=eye_sb[:], in_=eye_in[:])
        nc.scalar.dma_start(out=tri_sb[:], in_=tri_in[:])
        nc.scalar.dma_start(out=eye128_sb[:], in_=eye128_in[:])
        nc.scalar.dma_start(out=woT_sb[:], in_=woT_in[:])
        nc.scalar.dma_start(
            out=wv_sb[:].rearrange("p (ct m) -> p ct m", m=CSH),
            in_=wTv_in[:].rearrange("(ct p) m -> p ct m", p=128))
        # prefetch the remaining batch-0 x chunks right behind chunk 0
        def load_x(btc, eng=None):
            eng = eng or nc.sync
            xt = work.tile([128, CT * 512], bf16, tag="xtc", name=f"xtc{btc}",
                           bufs=5)
            x8t = work.tile([128, CT * 512], fp8, tag="x8", name=f"x8tc{btc}",
                            bufs=5)
            xr = xt[:].rearrange("p (ct f) -> p ct f", f=512)
            xi = xT_in[:, btc * 512:(btc + 1) * 512] \
                .rearrange("(ct p) f -> p ct f", p=128)
            eng.dma_start(out=xr[:, 0:4], in_=xi[:, 0:4])
            eng.dma_start(out=xr[:, 4:8], in_=xi[:, 4:8])
            eng.dma_start(
                out=x8t[:].rearrange("p (ct f) -> p ct f", f=512),
                in_=x8_in[:, btc * 512:(btc + 1) * 512]
                    .rearrange("(ct p) f -> p ct f", p=128))
            return xt, x8t

        xtc_pre = {btc: load_x(btc) for btc in range(1, QC - 1)}

        # PSUM: 2 (shared mm/aux) + 4 (fused scores x2) + 2 (AV accum) = 8 banks
        ps_x = outer.enter_context(tc.tile_pool(name="ps_x", bufs=2, space="PSUM"))
        ps_sc = outer.enter_context(tc.tile_pool(name="ps_sc", bufs=2, space="PSUM"))
        ps_av = outer.enter_context(tc.tile_pool(name="ps_av", bufs=2, space="PSUM"))

        big = outer.enter_context(tc.tile_pool(name="big", bufs=1))
        qrot = [big.tile([128, 512], bf16, name=f"qrot{i}") for i in range(BTC)]
        krot = [big.tile([128, 512], bf16, name=f"krot{i}") for i in range(BTC)]
        # vaug[b*HPC+h][cgrp]: [128, 4*VSTR]
        vaug = [[big.tile([128, 4 * VSTR], bf16, name=f"vaug{bh}_{cg}")
                 for cg in range(QC)] for bh in range(B * HPC)]
        attn_T = [big.tile([128, 512], bf16, name=f"attnT{i}") for i in range(BTC)]
        for bh in range(B * HPC):
            for cg in range(QC):
                ap65 = vaug[bh][cg][:].rearrange("p (kt e) -> p kt e", e=VSTR)
                nc.vector.memset(ap65[:, :, D:D + 1], 1.0)

        qkp = outer.enter_context(tc.tile_pool(name="qkp", bufs=6))
        ap_pool = outer.enter_context(tc.tile_pool(name="attn", bufs=8))
        nrm = outer.enter_context(tc.tile_pool(name="nrm", bufs=4))
        op = outer.enter_context(tc.tile_pool(name="outp", bufs=4))

        def prologue_chunk(b, cgrp, xtc, x8tc):
            """xT load + RMSNorm stats + QKV (deferred scale) + RoPE + v.

            Generator: yields between work quanta so the driver can interleave
            this chunk's emission with attention k-tiles.
            """
            btc = b * QC + cgrp
            tloc = slice(cgrp * 512, (cgrp + 1) * 512)
            # sum of squares over c (partitions): squares split across GPSIMD
            # and DVE, then a DoubleRow fp8 ones-matmul reduces two c-tiles
            # per pass (the pair dim is extra contraction). Chunk 0 is on the
            # startup critical path: bf16 squares (4x DVE mode) + plain bf16
            # reduction are lower-latency there even though they cost more PE.
            if btc == 0:
                xsq = work.tile([128, CT * 512], bf16, tag="xsqb",
                                name=f"xsq{btc}", bufs=1)
                for ct in range(CT):
                    cf = slice(ct * 512, (ct + 1) * 512)
                    nc.vector.tensor_tensor(out=xsq[:, cf], in0=xtc[:, cf],
                                            in1=xtc[:, cf], op=MUL)
                ssp = ps_x.tile([1, 512], f32, tag="mm", name=f"ssp{btc}")
                for ct in range(CT):
                    cf = slice(ct * 512, (ct + 1) * 512)
                    nc.tensor.matmul(ssp[:], onesb_bf[:], xsq[:, cf],
                                     start=(ct == 0), stop=(ct == CT - 1))
                yield
            else:
                xsq = work.tile([128, CT * 512], fp8, tag="xsq",
                                name=f"xsq{btc}", bufs=2)
                NPOOL = 3 if b == 0 else 2
                for ct in range(CT):
                    cf = slice(ct * 512, (ct + 1) * 512)
                    eng = nc.gpsimd if ct >= CT - NPOOL else nc.vector
                    eng.tensor_tensor(out=xsq[:, cf], in0=x8tc[:, cf],
                                      in1=x8tc[:, cf], op=MUL)
                ssp = ps_x.tile([1, 512], f32, tag="mm", name=f"ssp{btc}")
                xsq3 = xsq[:].rearrange("p (pr f) -> p pr f", f=1024)
                # dual-row ldweights needs a pair step that is a multiple of 16
                ones3 = onesb8[:].rearrange("p (i m) -> p i m", i=2)[:, :, 0:1]
                for k, pr in enumerate([0, 1, 2, 3]):
                    nc.tensor.matmul(
                        ssp[:], ones3,
                        xsq3[:, pr].rearrange("p (i f) -> p i f", f=512),
                        start=(k == 0), stop=(k == 3), perf_mode=DROW)
                yield
            # rsqrt(mean+eps) = exp(-0.5*ln(ss/C + eps)); ln+exp share one
            # activation table so the attention exps never force a reload.
            lnr = work.tile([1, 512], f32, tag="lnr", name=f"lnr{btc}")
            nc.scalar.activation(lnr[:], ssp[:], LN, scale=1.0 / C,
                                 bias=eps_sb[:])
            # the extra -6*ln2 bias folds the 1/64 weight-upscaling back out
            srow = work.tile([1, 512], f32, tag="srow", name=f"srow{btc}")
            nc.scalar.activation(srow[:], lnr[:], EXP, scale=-0.5,
                                 bias=m6ln2_sb[:])
            scaleB = work.tile([128, 512], f32, tag="scaleB",
                               name=f"scaleB{btc}")
            nc.gpsimd.partition_broadcast(scaleB[:], srow[:])
            yield
            yield
            # QKV chains and RoPE are ordered so the ps_x ring (bufs=2) always
            # has its previous-slot reader emitted before a slot is reused,
            # and PE quanta never sit directly behind a long latency chain.
            w3 = w_sb[:].rearrange("p (ct ft m) -> p ct ft m", ft=2, m=CSH)
            wv3 = w8v_sb[:].rearrange("p (blk i m) -> p blk i m", i=2, m=CSH)
            x83 = x8tc[:].rearrange("p (ct f) -> p ct f", f=512)

            def qkv_chain(ft):
                ps = ps_x.tile([128, 512], f32, tag="mm",
                               name=f"qkv{btc}_{ft}")
                if ft < 2:
                    for ct in range(CT):
                        nc.tensor.matmul(
                            ps[:], w3[:, ct, ft],
                            xtc[:, ct * 512:(ct + 1) * 512],
                            start=(ct == 0), stop=(ct == CT - 1))
                elif cgrp == 0:
                    # earliest tokens see little softmax averaging, so their
                    # V (which IS the output for token 0) stays bf16-exact
                    wv3b = wv_sb[:].rearrange("p (ct m) -> p ct m", m=CSH)
                    for ct in range(CT):
                        nc.tensor.matmul(
                            ps[:], wv3b[:, ct],
                            xtc[:, ct * 512:(ct + 1) * 512],
                            start=(ct == 0), stop=(ct == CT - 1))
                else:
                    # V in fp8 DoubleRow: quantization noise on V largely
                    # cancels in the softmax average for later tokens
                    for pr in range(CT // 2):
                        nc.tensor.matmul(
                            ps[:], wv3[:, pr],
                            x83[:, 2 * pr:2 * pr + 2, :],
                            start=(pr == 0), stop=(pr == CT // 2 - 1),
                            perf_mode=DROW)
                return ps

            def base_mul(ft, ps):
                base = qkp.tile([128, 512], bf16, tag="base",
                                name=f"base{btc}_{ft}")
                nc.vector.tensor_tensor(out=base[:], in0=ps[:],
                                        in1=scaleB[:], op=MUL)
                return base

            def rope(ft, base):
                psh = ps_x.tile([128, 512], f32, tag="mm",
                                name=f"psh{btc}_{ft}")
                nc.tensor.matmul(psh[:], perm_sb[:], base[:],
                                 start=True, stop=True)
                tmp = qkp.tile([128, 512], bf16, tag="tmp",
                               name=f"tmp{btc}_{ft}")
                nc.vector.tensor_tensor(out=tmp[:], in0=psh[:],
                                        in1=sin_sb[:, tloc], op=MUL)
                nc.vector.tensor_tensor(out=base[:], in0=base[:],
                                        in1=cos_sb[:, tloc], op=MUL)
                dst = qrot[btc] if ft == 0 else krot[btc]
                nc.vector.tensor_tensor(out=dst[:], in0=base[:],
                                        in1=tmp[:], op=ADD)

            ps_q = qkv_chain(0)
            yield
            ps_k = qkv_chain(1)
            yield
            base_q = base_mul(0, ps_q)
            yield
            yield
            ps_v = qkv_chain(2)
            yield
            base_k = base_mul(1, ps_k)
            yield
            rope(0, base_q)
            yield
            vtmp = qkp.tile([128, 512], bf16, tag="vtmp", name=f"vtmp{btc}")
            nc.vector.tensor_tensor(out=vtmp[:], in0=ps_v[:],
                                    in1=scaleB[:], op=MUL)
            yield
            rope(1, base_k)
            yield
            for h in range(HPC):
                hp = slice(h * 64, h * 64 + 64)
                va = vaug[b * HPC + h][cgrp]
                pvt = ps_x.tile([128, 4 * D], bf16, tag="mm",
                                name=f"vt{btc}_{h}")
                for ktl in range(4):
                    nc.tensor.transpose(
                        pvt[:, ktl * D:(ktl + 1) * D],
                        vtmp[hp, ktl * 128:(ktl + 1) * 128],
                        eye_sb[hp, :])
                nc.scalar.copy(
                    va[:].rearrange("p (kt e) -> p kt e", e=VSTR)[:, :, 0:D],
                    pvt[:].rearrange("p (kt e) -> p kt e", e=D))
                yield

        def attention_qc(b, qc, pump, last=False):
            """Causal attention for one 512-query chunk; both heads' scores
            live in one fused [128, 1024] PSUM tile (one exp per k-tile).
            Score matmuls run one k-tile ahead of AV so PE stays busy, and
            `pump` emits background (prologue/o_proj) quanta between k-tiles.
            """
            nkt = 4 * qc + 4
            avs = [ps_av.tile([D + 1, 512], f32, tag="av",
                              name=f"av{b}_{qc}_{h}") for h in range(HPC)]
            scs = {}

            def emit_sc(kt):
                cg, ktl = divmod(kt, 4)
                j = kt - 4 * qc
                n0 = 0 if j < 0 else j * 128
                kl = slice(ktl * 128, (ktl + 1) * 128)
                sc = ps_sc.tile([128, 1024], f32, tag="sc",
                                name=f"sc{b}_{qc}_{kt}")
                for h in range(HPC):
                    hp = slice(h * 64, h * 64 + 64)
                    nc.tensor.matmul(sc[:, h * 512 + n0:(h + 1) * 512],
                                     krot[b * QC + cg][hp, kl],
                                     qrot[b * QC + qc][hp, n0:512],
                                     start=True, stop=(j < 0))
                    if j >= 0:
                        # additive -30 causal bias on the diagonal block:
                        # masking happens before exp, off the critical path
                        nc.tensor.matmul(
                            sc[:, h * 512 + n0:h * 512 + n0 + 128],
                            eye128_sb[:], tri_sb[:], start=False, stop=True)
                scs[kt] = sc

            def emit_exp_av(kt):
                cg, ktl = divmod(kt, 4)
                j = kt - 4 * qc
                n0 = 0 if j < 0 else j * 128
                sc = scs.pop(kt)
                at = ap_pool.tile([128, 1024], bf16, tag="at",
                                  name=f"at{b}_{qc}_{kt}")
                sc3 = sc[:].rearrange("p (h q) -> p h q", h=2)
                at3 = at[:].rearrange("p (h q) -> p h q", h=2)
                nc.scalar.activation(at3[:, :, n0:512], sc3[:, :, n0:512], EXP,
                                     scale=0.125)
                for h in range(HPC):
                    nc.tensor.matmul(
                        avs[h][:, n0:512],
                        vaug[b * HPC + h][cg][:, ktl * VSTR: ktl * VSTR + D + 1],
                        at[:, h * 512 + n0:(h + 1) * 512],
                        start=(kt == 0), stop=(kt == nkt - 1))

            emit_sc(0)
            for kt in range(nkt):
                if kt + 1 < nkt:
                    emit_sc(kt + 1)
                emit_exp_av(kt)
                pump(1)
            for h in range(HPC):
                inv = nrm.tile([1, 512], bf16, tag="inv", name=f"inv{b}_{qc}_{h}")
                nc.vector.reciprocal(inv[:], avs[h][D:D + 1, :])
                if last:
                    # tail: PE is idle here and the broadcast is on the
                    # critical path, so use the low-latency PE outer product
                    bcp = ps_x.tile([64, 512], f32, tag="mm",
                                    name=f"bc{b}_{qc}_{h}")
                    nc.tensor.matmul(bcp[:], ones64_bf[:], inv[:],
                                     start=True, stop=True)
                    bcs = nrm.tile([64, 512], f32, tag="bcs",
                                   name=f"bcs{b}_{qc}_{h}")
                    nc.scalar.copy(bcs[:], bcp[:])
                    nc.vector.tensor_tensor(
                        out=attn_T[b * QC + qc][h * 64:(h + 1) * 64, :],
                        in0=avs[h][0:D, :], in1=bcs[:], op=MUL)
                else:
                    invB = nrm.tile([64, 512], bf16, tag="invB",
                                    name=f"invB{b}_{qc}_{h}")
                    nc.gpsimd.partition_broadcast(invB[:], inv[:])
                    nc.vector.tensor_tensor(
                        out=attn_T[b * QC + qc][h * 64:(h + 1) * 64, :],
                        in0=avs[h][0:D, :], in1=invB[:], op=MUL)
                pump(2)

        def oproj_qc(b, qc, split, tail=False):
            """split=True routes half the PSUM evacuation to ACT (used when
            pumped during a phase where DVE is the busier engine); tail=True
            fires per-half stores on both DMA queues to shorten the drain."""
            btc = b * QC + qc
            for jj in range(4):
                i = btc * 4 + jj
                ob = op.tile([128, C], bf16, tag="ob", name=f"ob{i}")
                for half in range(2):
                    # at the tail the score PSUM banks are free, so half 1
                    # borrows the ps_sc pool: four po tiles in flight lets
                    # the DVE/ACT evacuations pipeline with nothing serial
                    pool = ps_sc if tail and half == 1 else ps_x
                    tag = "sc" if tail and half == 1 else "mm"
                    po = pool.tile([128, 512], f32, tag=tag,
                                   name=f"po{i}_{half}")
                    nc.tensor.matmul(po[:],
                                     attn_T[btc][:, jj * 128:(jj + 1) * 128],
                                     woT_sb[:, half * 512:(half + 1) * 512],
                                     start=True, stop=True)
                    if (split or tail) and half == 1:
                        nc.scalar.copy(ob[:, half * 512:(half + 1) * 512],
                                       po[:])
                    else:
                        nc.vector.tensor_copy(
                            ob[:, half * 512:(half + 1) * 512], po[:])
                nc.sync.dma_start(out=out_dram[i * 128:(i + 1) * 128, :],
                                  in_=ob[:])
                yield

        # -- emission schedule: b0 prologue, then attention with background --
        bg = []

        def pump(n=1):
            done = 0
            while bg and done < n:
                try:
                    next(bg[0])
                    done += 1
                except StopIteration:
                    bg.pop(0)

        def run_all(gen):
            for _ in gen:
                pass

        # batch-0 prologues run with a staggered 2-chunk software pipeline so
        # chunk n+1's stats chain overlaps chunk n's QKV/RoPE on each engine
        # chunks 0-2 are emitted eagerly; chunk 3 joins the background queue
        # so attention(0,0) starts one chunk earlier and chunk 3's QKV/RoPE
        # fills the ACT-bound attention gaps
        run_all(prologue_chunk(0, 0, xtc0, x8tc0))
        run_all(prologue_chunk(0, 1, *xtc_pre[1]))
        run_all(prologue_chunk(0, 2, *xtc_pre[2]))
        bg.append(prologue_chunk(0, 3, *load_x(3)))

        b1_x = {0: load_x(QC)}
        for qc in range(QC):
            # batch-1 x chunks are loaded one q-chunk ahead so the pre-emitted
            # stats chain below never waits on an in-flight DMA
            if qc + 1 < QC:
                b1_x[qc + 1] = load_x(QC + qc + 1)
            bg.append(prologue_chunk(1, qc, *b1_x.pop(qc)))
            if qc > 0:
                bg.append(oproj_qc(0, qc - 1, split=True))
            attention_qc(0, qc, pump)
        bg.append(oproj_qc(0, QC - 1, split=False))
        # batch 1 ends on the smallest q-chunk so the drain tail is short
        b1_order = [0, 1, 2, 3]
        for i, qc in enumerate(b1_order):
            if i > 0:
                bg.append(oproj_qc(1, b1_order[i - 1], split=False))
            attention_qc(1, qc, pump, last=(i == len(b1_order) - 1))
        pump(10 ** 6)
        run_all(oproj_qc(1, b1_order[-1], split=True, tail=True))

    nc.compile()
    return nc


FP8 = ml_dtypes.float8_e4m3


def _prep_inputs(x, w_qkv, rms_w):
    cosT, sinT, tri, eye, eye128, perm = _host_tables()
    xf = np.asarray(x, dtype=np.float32).reshape(BT, C)
    xT = np.ascontiguousarray(xf.T)
    w = np.asarray(w_qkv, dtype=np.float32)
    rw = np.asarray(rms_w, dtype=np.float32)
    in_maps = []
    for i in range(NCORES):
        rows = slice(i * CSH, (i + 1) * CSH)
        # weights are scaled x64 (fp8 e4m3 normal range for V; harmless in
        # bf16 for Q/K); the 1/64 is folded back via the rsqrt row, and
        # 1/sqrt(D) via the exp input scale on-device
        wq = w[0 * C:1 * C][rows] * rw[None, :]
        wk = w[1 * C:2 * C][rows] * rw[None, :]
        wv = w[2 * C:3 * C][rows] * rw[None, :] * 64.0
        wT = np.concatenate([wq, wk], axis=0).T * 64.0   # [C, 2*CSH]
        # V -> [p, (P, i, m)] with c = P*256 + i*128 + p
        w8v = wv.reshape(CSH, CT // 2, 2, 128).transpose(3, 1, 2, 0)
        w8v = np.ascontiguousarray(w8v.reshape(128, -1))
        in_maps.append({
            "xT": xT.astype(BF16), "x8": xT.astype(FP8),
            "wT": np.ascontiguousarray(wT).astype(BF16),
            "w8v": w8v.astype(FP8),
            "wTv": np.ascontiguousarray(wv.T).astype(BF16),
            "cosT": cosT, "sinT": sinT, "tri": tri, "eye": eye,
            "eye128": eye128, "perm": perm,
        })
    return in_maps


def kernel(x, attention_mask, w_qkv, b_qkv, w_o, b_o, rms_w):
    from concourse.bass_utils import run_bass_kernel_spmd

    if "nc" not in _cache:
        _cache["nc"] = _build()
    nc = _cache["nc"]

    in_maps = _prep_inputs(x, w_qkv, rms_w)
    wo = np.asarray(w_o, dtype=np.float32)
    for i in range(NCORES):
        cols = slice(i * CSH, (i + 1) * CSH)
        in_maps[i]["woT"] = np.ascontiguousarray(wo[:, cols].T).astype(BF16)

    res = run_bass_kernel_spmd(nc, in_maps, core_ids=list(range(NCORES)))

    acc = np.zeros((BT, C), dtype=np.float32)
    for i in range(NCORES):
        acc += res.results[i]["out"].astype(np.float32)
    acc += np.asarray(b_o, dtype=np.float32)[None, :]
    return acc.reshape(B, T, C)


# revision 180
# speedup vs baseline: 1.0014x; 1.0014x over previous
"""Trainium2 Bass kernel for an attention block (RMSNorm + fused QKV + RoPE +
causal MHA + output projection), Megatron-style head sharding over 8 NeuronCores.

Shapes (hardcoded): B=2, T=2048, C=1024, H=16, D=64. Each core handles 2 heads.

Final design (235.5us baseline -> 164.0us):
- engine balance: PE matmuls / DVE elementwise+PSUM evac / ACT exp+copies /
  GPSIMD squares+partition-broadcasts / two HWDGE DMA queues (SP=x-stream,
  ACT=constants)
- rsqrt via ACT ln -> exp(-0.5x - 6ln2) with a manually preloaded table that
  covers both Ln and Exp (zero activation-table reloads); the -6ln2 bias
  undoes the x64 weight upscaling used for fp8
- V-projection and the sum-of-squares reduction run in fp8e4m3 DoubleRow
  (2 contraction rows/partition, half PE cost); Q/K stay bf16 because score
  noise through the softmax is the dominant error path; V of the first 512
  tokens per batch stays bf16 since early tokens get no softmax averaging
- both heads' scores packed in one [128,1024] 2-bank PSUM tile, single fused
  exp per k-tile with scale=1/8 (the 1/sqrt(D) fold); causal masking is a
  -240 additive bias matmul into PSUM before the exp (off the critical path)
- score matmuls emitted one k-tile ahead of AV so PE never waits on exp
- softmax denominator from an augmented [v | 1] AV matmul row; normalize via
  DVE reciprocal + GPSIMD partition-broadcast (PE outer product on the tail)
- emission interleaving: batch-1 prologue and previous-chunk o_proj quanta
  are pumped between attention k-tiles to fill the in-order PE stream; PSUM
  lives in a disciplined 2-slot ring (readers always emitted before reuse)

Host: shards weights (x64 into fp8/bf16 layouts incl. the dual-row V blocks),
precomputes RoPE tables / causal bias / identities, sums the 8 partial
outputs in fp32, adds b_o. b_qkv supported only as zeros (spec fill=zeros).
"""

import numpy as np
import ml_dtypes

B, T, C, H, D = 2, 2048, 1024, 16, 64
BT = B * T
NCORES = 8
HPC = H // NCORES               # heads per core = 2
CSH = HPC * D                   # per-core attention channels = 128
EPS = 1e-5
ROPE_BASE = 10000.0

CT = C // 128                   # 8 c-tiles
BTC = BT // 512                 # 8 bt chunks of 512
QC = T // 512                   # 4 q chunks of 512 per batch
VSTR = 80                       # per-ktile stride in v_aug (32B aligned)

BF16 = ml_dtypes.bfloat16

_cache = {}


def _host_tables():
    half = D // 2
    inv_freq = 1.0 / (ROPE_BASE ** (np.arange(half, dtype=np.float64) / half))
    t = np.arange(T, dtype=np.float64)
    ang = t[None, :] * inv_freq[:, None]
    ang = np.concatenate([ang, ang], axis=0)      # [64, T]
    cos = np.cos(ang)
    sin = np.sin(ang)
    sgn = np.where(np.arange(D) < half, -1.0, 1.0)[:, None]
    sinS = sin * sgn
    cosT = np.tile(cos, (2, 1)).astype(BF16)      # [128, T]
    sinT = np.tile(sinS, (2, 1)).astype(BF16)
    # additive causal bias for the diagonal 128x128 score blocks; the exp
    # applies scale=1/8 (the 1/sqrt(D) fold), so -240 lands at exp(-30)~=0
    tri = np.where(np.arange(128)[:, None] <= np.arange(128)[None, :],
                   0.0, -240.0).astype(BF16)
    eye = np.concatenate([np.eye(D), np.eye(D)], axis=0).astype(BF16)  # [128,D]
    eye128 = np.eye(128, dtype=BF16)
    sh = np.r_[np.arange(32, 64), np.arange(0, 32),
               np.arange(96, 128), np.arange(64, 96)]
    perm = np.zeros((128, 128), dtype=BF16)
    perm[sh, np.arange(128)] = 1.0    # lhsT[s, p] = 1 iff s = sh(p)
    return cosT, sinT, tri, eye, eye128, perm


def _build():
    import concourse.bacc as bacc
    import concourse.mybir as mybir
    from concourse.tile import TileContext
    from contextlib import ExitStack

    f32 = mybir.dt.float32
    bf16 = mybir.dt.bfloat16
    fp8 = mybir.dt.float8e4
    DROW = mybir.MatmulPerfMode.DoubleRow
    MUL = mybir.AluOpType.mult
    ADD = mybir.AluOpType.add
    EXP = mybir.ActivationFunctionType.Exp
    LN = mybir.ActivationFunctionType.Ln

    nc = bacc.Bacc("TRN2", target_bir_lowering=False, debug=False,
                   num_devices=NCORES)

    xT_in = nc.dram_tensor("xT", [C, BT], bf16, kind="ExternalInput").ap()
    x8_in = nc.dram_tensor("x8", [C, BT], fp8, kind="ExternalInput").ap()
    wT_in = nc.dram_tensor("wT", [C, 2 * CSH], bf16, kind="ExternalInput").ap()
    # w8v layout: [p, (pair P, i, m)] — each P slice is a contiguous [2, 128]
    # dual-row block as the fp8 Ldweights ISA requires
    w8v_in = nc.dram_tensor("w8v", [128, (CT // 2) * 2 * CSH], fp8,
                            kind="ExternalInput").ap()
    wTv_in = nc.dram_tensor("wTv", [C, CSH], bf16, kind="ExternalInput").ap()
    woT_in = nc.dram_tensor("woT", [CSH, C], bf16, kind="ExternalInput").ap()
    cos_in = nc.dram_tensor("cosT", [128, T], bf16, kind="ExternalInput").ap()
    sin_in = nc.dram_tensor("sinT", [128, T], bf16, kind="ExternalInput").ap()
    tri_in = nc.dram_tensor("tri", [128, 128], bf16, kind="ExternalInput").ap()
    eye_in = nc.dram_tensor("eye", [128, D], bf16, kind="ExternalInput").ap()
    eye128_in = nc.dram_tensor("eye128", [128, 128], bf16,
                               kind="ExternalInput").ap()
    perm_in = nc.dram_tensor("perm", [128, 128], bf16, kind="ExternalInput").ap()
    out_dram = nc.dram_tensor("out", [BT, C], bf16, kind="ExternalOutput").ap()

    with nc.allow_low_precision(reason="fp32r broadcast operands are exact for 1.0*x"), \
         TileContext(nc) as tc, ExitStack() as outer:
        cpool = outer.enter_context(tc.tile_pool(name="consts", bufs=1))
        work = outer.enter_context(tc.tile_pool(name="work", bufs=3))

        # first x chunk DMA goes out before the big constant loads so the
        # pipeline starts immediately
        xtc0 = work.tile([128, CT * 512], bf16, tag="xtc", name="xtc0", bufs=5)
        x8tc0 = work.tile([128, CT * 512], fp8, tag="x8", name="x8tc0", bufs=5)
        x0r = xtc0[:].rearrange("p (ct f) -> p ct f", f=512)
        xi0 = xT_in[:, 0:512].rearrange("(ct p) f -> p ct f", p=128)
        nc.sync.dma_start(out=x0r[:, 0:4], in_=xi0[:, 0:4])
        nc.sync.dma_start(out=x0r[:, 4:8], in_=xi0[:, 4:8])

        w_sb = cpool.tile([128, CT * 2 * CSH], bf16)
        w8v_sb = cpool.tile([128, (CT // 2) * 2 * CSH], fp8)
        wv_sb = cpool.tile([128, CT * CSH], bf16)
        woT_sb = cpool.tile([128, C], bf16)
        tri_sb = cpool.tile([128, 128], bf16)
        eye_sb = cpool.tile([128, D], bf16)
        eye128_sb = cpool.tile([128, 128], bf16)
        perm_sb = cpool.tile([128, 128], bf16)
        onesb8 = cpool.tile([128, 32], fp8)
        onesb_bf = cpool.tile([128, 1], bf16)
        ones1_f32 = cpool.tile([1, 128], f32)
        ones64_bf = cpool.tile([1, 64], bf16)
        eps_sb = cpool.tile([1, 1], f32)
        m6ln2_sb = cpool.tile([1, 1], f32)
        cos_sb = cpool.tile([128, T], bf16)
        sin_sb = cpool.tile([128, T], bf16)
        nc.vector.memset(onesb8[:], 1.0)
        nc.vector.memset(onesb_bf[:], 1.0)
        nc.vector.memset(ones1_f32[:], 1.0)
        nc.vector.memset(ones64_bf[:], 1.0)
        nc.vector.memset(eps_sb[:], EPS)
        nc.vector.memset(m6ln2_sb[:], -6.0 * float(np.log(2.0)))
        # preload the one activation table that covers both Ln and Exp so the
        # compiler's table-load pass never inserts a reload
        nc.scalar.add_instruction(mybir.InstLoadActFuncSet(
            name=nc.get_next_instruction_name(), ins=[], outs=[],
            act_func_set_id=6))
        # SP HWDGE queue carries only the x-chunk stream (w first, it gates
        # the first QKV); everything else rides the idle ACT HWDGE queue
        nc.sync.dma_start(
            out=w_sb[:].rearrange("p (ct f) -> p ct f", f=2 * CSH),
            in_=wT_in[:].rearrange("(ct p) f -> p ct f", p=128))
        nc.sync.dma_start(
            out=x8tc0[:].rearrange("p (ct f) -> p ct f", f=512),
            in_=x8_in[:, 0:512].rearrange("(ct p) f -> p ct f", p=128))
        nc.sync.dma_start(out=w8v_sb[:], in_=w8v_in[:])
        nc.scalar.dma_start(out=cos_sb[:], in_=cos_in[:])
        nc.scalar.dma_start(out=sin_sb[:], in_=sin_in[:])
        nc.scalar.dma_start(out=perm_sb[:], in_=perm_in[:])
        nc.scalar.dma_start(out=eye_sb[:], in_=eye_in[:])
        nc.scalar.dma_start(out=tri_sb[:], in_=tri_in[:])
        nc.scalar.dma_start(out=eye128_sb[:], in_=eye128_in[:])
        nc.scalar.dma_start(out=woT_sb[:], in_=woT_in[:])
        nc.scalar.dma_start(
            out=wv_sb[:].rearrange("p (ct m) -> p ct m", m=CSH),
            in_=wTv_in[:].rearrange("(ct p) m -> p ct m", p=128))
        # prefetch the remaining batch-0 x chunks right behind chunk 0
        def load_x(btc, eng=None):
            eng = eng or nc.sync
            xt = work.tile([128, CT * 512], bf16, tag="xtc", name=f"xtc{btc}",
                           bufs=5)
            x8t = work.tile([128, CT * 512], fp8, tag="x8", name=f"x8tc{btc}",
                            bufs=5)
            xr = xt[:].rearrange("p (ct f) -> p ct f", f=512)
            xi = xT_in[:, btc * 512:(btc + 1) * 512] \
                .rearrange("(ct p) f -> p ct f", p=128)
            eng.dma_start(out=xr[:, 0:4], in_=xi[:, 0:4])
            eng.dma_start(out=xr[:, 4:8], in_=xi[:, 4:8])
            eng.dma_start(
                out=x8t[:].rearrange("p (ct f) -> p ct f", f=512),
                in_=x8_in[:, btc * 512:(btc + 1) * 512]
                    .rearrange("(ct p) f -> p ct f", p=128))
            return xt, x8t

        xtc_pre = {btc: load_x(btc) for btc in range(1, QC - 1)}

        # PSUM: 2 (shared mm/aux) + 4 (fused scores x2) + 2 (AV accum) = 8 banks
        ps_x = outer.enter_context(tc.tile_pool(name="ps_x", bufs=2, space="PSUM"))
        ps_sc = outer.enter_context(tc.tile_pool(name="ps_sc", bufs=2, space="PSUM"))
        ps_av = outer.enter_context(tc.tile_pool(name="ps_av", bufs=2, space="PSUM"))

        big = outer.enter_context(tc.tile_pool(name="big", bufs=1))
        qrot = [big.tile([128, 512], bf16, name=f"qrot{i}") for i in range(BTC)]
        krot = [big.tile([128, 512], bf16, name=f"krot{i}") for i in range(BTC)]
        # vaug[b*HPC+h][cgrp]: [128, 4*VSTR]
        vaug = [[big.tile([128, 4 * VSTR], bf16, name=f"vaug{bh}_{cg}")
                 for cg in range(QC)] for bh in range(B * HPC)]
        attn_T = [big.tile([128, 512], bf16, name=f"attnT{i}") for i in range(BTC)]
        for bh in range(B * HPC):
            for cg in range(QC):
                ap65 = vaug[bh][cg][:].rearrange("p (kt e) -> p kt e", e=VSTR)
                nc.vector.memset(ap65[:, :, D:D + 1], 1.0)

        qkp = outer.enter_context(tc.tile_pool(name="qkp", bufs=6))
        ap_pool = outer.enter_context(tc.tile_pool(name="attn", bufs=8))
        nrm = outer.enter_context(tc.tile_pool(name="nrm", bufs=3))
        op = outer.enter_context(tc.tile_pool(name="outp", bufs=4))

        def prologue_chunk(b, cgrp, xtc, x8tc):
            """xT load + RMSNorm stats + QKV (deferred scale) + RoPE + v.

            Generator: yields between work quanta so the driver can interleave
            this chunk's emission with attention k-tiles.
            """
            btc = b * QC + cgrp
            tloc = slice(cgrp * 512, (cgrp + 1) * 512)
            # sum of squares over c (partitions): squares split across GPSIMD
            # and DVE, then a DoubleRow fp8 ones-matmul reduces two c-tiles
            # per pass (the pair dim is extra contraction). Chunk 0 is on the
            # startup critical path: bf16 squares (4x DVE mode) + plain bf16
            # reduction are lower-latency there even though they cost more PE.
            if btc == 0:
                xsq = work.tile([128, CT * 512], bf16, tag="xsqb",
                                name=f"xsq{btc}", bufs=1)
                for ct in range(CT):
                    cf = slice(ct * 512, (ct + 1) * 512)
                    nc.vector.tensor_tensor(out=xsq[:, cf], in0=xtc[:, cf],
                                            in1=xtc[:, cf], op=MUL)
                ssp = ps_x.tile([1, 512], f32, tag="mm", name=f"ssp{btc}")
                for ct in range(CT):
                    cf = slice(ct * 512, (ct + 1) * 512)
                    nc.tensor.matmul(ssp[:], onesb_bf[:], xsq[:, cf],
                                     start=(ct == 0), stop=(ct == CT - 1))
                yield
            else:
                xsq = work.tile([128, CT * 512], fp8, tag="xsq",
                                name=f"xsq{btc}", bufs=3)
                NPOOL = 3 if b == 0 else 2
                for ct in range(CT):
                    cf = slice(ct * 512, (ct + 1) * 512)
                    eng = nc.gpsimd if ct >= CT - NPOOL else nc.vector
                    eng.tensor_tensor(out=xsq[:, cf], in0=x8tc[:, cf],
                                      in1=x8tc[:, cf], op=MUL)
                ssp = ps_x.tile([1, 512], f32, tag="mm", name=f"ssp{btc}")
                xsq3 = xsq[:].rearrange("p (pr f) -> p pr f", f=1024)
                # dual-row ldweights needs a pair step that is a multiple of 16
                ones3 = onesb8[:].rearrange("p (i m) -> p i m", i=2)[:, :, 0:1]
                for k, pr in enumerate([0, 1, 2, 3]):
                    nc.tensor.matmul(
                        ssp[:], ones3,
                        xsq3[:, pr].rearrange("p (i f) -> p i f", f=512),
                        start=(k == 0), stop=(k == 3), perf_mode=DROW)
                yield
            # rsqrt(mean+eps) = exp(-0.5*ln(ss/C + eps)); ln+exp share one
            # activation table so the attention exps never force a reload.
            lnr = work.tile([1, 512], f32, tag="lnr", name=f"lnr{btc}")
            nc.scalar.activation(lnr[:], ssp[:], LN, scale=1.0 / C,
                                 bias=eps_sb[:])
            # the extra -6*ln2 bias folds the 1/64 weight-upscaling back out
            srow = work.tile([1, 512], f32, tag="srow", name=f"srow{btc}")
            nc.scalar.activation(srow[:], lnr[:], EXP, scale=-0.5,
                                 bias=m6ln2_sb[:])
            scaleB = work.tile([128, 512], f32, tag="scaleB",
                               name=f"scaleB{btc}")
            nc.gpsimd.partition_broadcast(scaleB[:], srow[:])
            yield
            yield
            # QKV chains and RoPE are ordered so the ps_x ring (bufs=2) always
            # has its previous-slot reader emitted before a slot is reused,
            # and PE quanta never sit directly behind a long latency chain.
            w3 = w_sb[:].rearrange("p (ct ft m) -> p ct ft m", ft=2, m=CSH)
            wv3 = w8v_sb[:].rearrange("p (blk i m) -> p blk i m", i=2, m=CSH)
            x83 = x8tc[:].rearrange("p (ct f) -> p ct f", f=512)

            def qkv_chain(ft):
                ps = ps_x.tile([128, 512], f32, tag="mm",
                               name=f"qkv{btc}_{ft}")
                if ft < 2:
                    for ct in range(CT):
                        nc.tensor.matmul(
                            ps[:], w3[:, ct, ft],
                            xtc[:, ct * 512:(ct + 1) * 512],
                            start=(ct == 0), stop=(ct == CT - 1))
                elif cgrp == 0:
                    # earliest tokens see little softmax averaging, so their
                    # V (which IS the output for token 0) stays bf16-exact
                    wv3b = wv_sb[:].rearrange("p (ct m) -> p ct m", m=CSH)
                    for ct in range(CT):
                        nc.tensor.matmul(
                            ps[:], wv3b[:, ct],
                            xtc[:, ct * 512:(ct + 1) * 512],
                            start=(ct == 0), stop=(ct == CT - 1))
                else:
                    # V in fp8 DoubleRow: quantization noise on V largely
                    # cancels in the softmax average for later tokens
                    for pr in range(CT // 2):
                        nc.tensor.matmul(
                            ps[:], wv3[:, pr],
                            x83[:, 2 * pr:2 * pr + 2, :],
                            start=(pr == 0), stop=(pr == CT // 2 - 1),
                            perf_mode=DROW)
                return ps

            def base_mul(ft, ps):
                base = qkp.tile([128, 512], bf16, tag="base",
                                name=f"base{btc}_{ft}")
                nc.vector.tensor_tensor(out=base[:], in0=ps[:],
                                        in1=scaleB[:], op=MUL)
                return base

            def rope(ft, base):
                psh = ps_x.tile([128, 512], f32, tag="mm",
                                name=f"psh{btc}_{ft}")
                nc.tensor.matmul(psh[:], perm_sb[:], base[:],
                                 start=True, stop=True)
                tmp = qkp.tile([128, 512], bf16, tag="tmp",
                               name=f"tmp{btc}_{ft}")
                nc.vector.tensor_tensor(out=tmp[:], in0=psh[:],
                                        in1=sin_sb[:, tloc], op=MUL)
                nc.vector.tensor_tensor(out=base[:], in0=base[:],
                                        in1=cos_sb[:, tloc], op=MUL)
                dst = qrot[btc] if ft == 0 else krot[btc]
                nc.vector.tensor_tensor(out=dst[:], in0=base[:],
                                        in1=tmp[:], op=ADD)

            ps_q = qkv_chain(0)
            yield
            ps_k = qkv_chain(1)
            yield
            base_q = base_mul(0, ps_q)
            yield
            yield
            ps_v = qkv_chain(2)
            yield
            base_k = base_mul(1, ps_k)
            yield
            rope(0, base_q)
            yield
            vtmp = qkp.tile([128, 512], bf16, tag="vtmp", name=f"vtmp{btc}")
            nc.vector.tensor_tensor(out=vtmp[:], in0=ps_v[:],
                                    in1=scaleB[:], op=MUL)
            yield
            rope(1, base_k)
            yield
            for h in range(HPC):
                hp = slice(h * 64, h * 64 + 64)
                va = vaug[b * HPC + h][cgrp]
                pvt = ps_x.tile([128, 4 * D], bf16, tag="mm",
                                name=f"vt{btc}_{h}")
                for ktl in range(4):
                    nc.tensor.transpose(
                        pvt[:, ktl * D:(ktl + 1) * D],
                        vtmp[hp, ktl * 128:(ktl + 1) * 128],
                        eye_sb[hp, :])
                nc.scalar.copy(
                    va[:].rearrange("p (kt e) -> p kt e", e=VSTR)[:, :, 0:D],
                    pvt[:].rearrange("p (kt e) -> p kt e", e=D))
                yield

        def attention_qc(b, qc, pump, last=False):
            """Causal attention for one 512-query chunk; both heads' scores
            live in one fused [128, 1024] PSUM tile (one exp per k-tile).
            Score matmuls run one k-tile ahead of AV so PE stays busy, and
            `pump` emits background (prologue/o_proj) quanta between k-tiles.
            """
            nkt = 4 * qc + 4
            avs = [ps_av.tile([D + 1, 512], f32, tag="av",
                              name=f"av{b}_{qc}_{h}") for h in range(HPC)]
            scs = {}

            def emit_sc(kt):
                cg, ktl = divmod(kt, 4)
                j = kt - 4 * qc
                n0 = 0 if j < 0 else j * 128
                kl = slice(ktl * 128, (ktl + 1) * 128)
                sc = ps_sc.tile([128, 1024], f32, tag="sc",
                                name=f"sc{b}_{qc}_{kt}")
                for h in range(HPC):
                    hp = slice(h * 64, h * 64 + 64)
                    nc.tensor.matmul(sc[:, h * 512 + n0:(h + 1) * 512],
                                     krot[b * QC + cg][hp, kl],
                                     qrot[b * QC + qc][hp, n0:512],
                                     start=True, stop=(j < 0))
                    if j >= 0:
                        # additive -30 causal bias on the diagonal block:
                        # masking happens before exp, off the critical path
                        nc.tensor.matmul(
                            sc[:, h * 512 + n0:h * 512 + n0 + 128],
                            eye128_sb[:], tri_sb[:], start=False, stop=True)
                scs[kt] = sc

            def emit_exp_av(kt):
                cg, ktl = divmod(kt, 4)
                j = kt - 4 * qc
                n0 = 0 if j < 0 else j * 128
                sc = scs.pop(kt)
                at = ap_pool.tile([128, 1024], bf16, tag="at",
                                  name=f"at{b}_{qc}_{kt}")
                sc3 = sc[:].rearrange("p (h q) -> p h q", h=2)
                at3 = at[:].rearrange("p (h q) -> p h q", h=2)
                nc.scalar.activation(at3[:, :, n0:512], sc3[:, :, n0:512], EXP,
                                     scale=0.125)
                for h in range(HPC):
                    nc.tensor.matmul(
                        avs[h][:, n0:512],
                        vaug[b * HPC + h][cg][:, ktl * VSTR: ktl * VSTR + D + 1],
                        at[:, h * 512 + n0:(h + 1) * 512],
                        start=(kt == 0), stop=(kt == nkt - 1))

            emit_sc(0)
            for kt in range(nkt):
                if kt + 1 < nkt:
                    emit_sc(kt + 1)
                emit_exp_av(kt)
                pump(1)
            for h in range(HPC):
                inv = nrm.tile([1, 512], bf16, tag="inv", name=f"inv{b}_{qc}_{h}")
                nc.vector.reciprocal(inv[:], avs[h][D:D + 1, :])
                if last:
                    # tail: PE is idle here and the broadcast is on the
                    # critical path, so use the low-latency PE outer product
                    bcp = ps_x.tile([64, 512], f32, tag="mm",
                                    name=f"bc{b}_{qc}_{h}")
                    nc.tensor.matmul(bcp[:], ones64_bf[:], inv[:],
                                     start=True, stop=True)
                    bcs = nrm.tile([64, 512], f32, tag="bcs",
                                   name=f"bcs{b}_{qc}_{h}")
                    nc.scalar.copy(bcs[:], bcp[:])
                    nc.vector.tensor_tensor(
                        out=attn_T[b * QC + qc][h * 64:(h + 1) * 64, :],
                        in0=avs[h][0:D, :], in1=bcs[:], op=MUL)
                else:
                    invB = nrm.tile([64, 512], bf16, tag="invB",
                                    name=f"invB{b}_{qc}_{h}")
                    nc.gpsimd.partition_broadcast(invB[:], inv[:])
                    nc.vector.tensor_tensor(
                        out=attn_T[b * QC + qc][h * 64:(h + 1) * 64, :],
                        in0=avs[h][0:D, :], in1=invB[:], op=MUL)
                pump(2)

        def oproj_qc(b, qc, split, tail=False):
            """split=True routes half the PSUM evacuation to ACT (used when
            pumped during a phase where DVE is the busier engine); tail=True
            fires per-half stores on both DMA queues to shorten the drain."""
            btc = b * QC + qc
            for jj in range(4):
                i = btc * 4 + jj
                ob = op.tile([128, C], bf16, tag="ob", name=f"ob{i}")
                for half in range(2):
                    # at the tail the score PSUM banks are free, so half 1
                    # borrows the ps_sc pool: four po tiles in flight lets
                    # the DVE/ACT evacuations pipeline with nothing serial
                    pool = ps_sc if tail and half == 1 else ps_x
                    tag = "sc" if tail and half == 1 else "mm"
                    po = pool.tile([128, 512], f32, tag=tag,
                                   name=f"po{i}_{half}")
                    nc.tensor.matmul(po[:],
                                     attn_T[btc][:, jj * 128:(jj + 1) * 128],
                                     woT_sb[:, half * 512:(half + 1) * 512],
                                     start=True, stop=True)
                    if (split or tail) and half == 1:
                        nc.scalar.copy(ob[:, half * 512:(half + 1) * 512],
                                       po[:])
                    else:
                        nc.vector.tensor_copy(
                            ob[:, half * 512:(half + 1) * 512], po[:])
                nc.sync.dma_start(out=out_dram[i * 128:(i + 1) * 128, :],
                                  in_=ob[:])
                yield

        # -- emission schedule: b0 prologue, then attention with background --
        bg = []

        def pump(n=1):
            done = 0
            while bg and done < n:
                try:
                    next(bg[0])
                    done += 1
                except StopIteration:
                    bg.pop(0)

        def run_all(gen):
            for _ in gen:
                pass

        # batch-0 prologues run with a staggered 2-chunk software pipeline so
        # chunk n+1's stats chain overlaps chunk n's QKV/RoPE on each engine
        # chunks 0-2 are emitted eagerly; chunk 3 joins the background queue
        # so attention(0,0) starts one chunk earlier and chunk 3's QKV/RoPE
        # fills the ACT-bound attention gaps
        run_all(prologue_chunk(0, 0, xtc0, x8tc0))
        run_all(prologue_chunk(0, 1, *xtc_pre[1]))
        run_all(prologue_chunk(0, 2, *xtc_pre[2]))
        bg.append(prologue_chunk(0, 3, *load_x(3)))

        b1_x = {0: load_x(QC)}
        for qc in range(QC):
            # batch-1 x chunks are loaded one q-chunk ahead so the pre-emitted
            # stats chain below never waits on an in-flight DMA
            if qc + 1 < QC:
                b1_x[qc + 1] = load_x(QC + qc + 1)
            bg.append(prologue_chunk(1, qc, *b1_x.pop(qc)))
            if qc > 0:
                bg.append(oproj_qc(0, qc - 1, split=True))
            attention_qc(0, qc, pump)
        bg.append(oproj_qc(0, QC - 1, split=False))
        # batch 1 ends on the smallest q-chunk so the drain tail is short
        b1_order = [0, 1, 2, 3]
        for i, qc in enumerate(b1_order):
            if i > 0:
                bg.append(oproj_qc(1, b1_order[i - 1], split=False))
            attention_qc(1, qc, pump, last=(i == len(b1_order) - 1))
        pump(10 ** 6)
        run_all(oproj_qc(1, b1_order[-1], split=True, tail=True))

    nc.compile()
    return nc


FP8 = ml_dtypes.float8_e4m3


def _prep_inputs(x, w_qkv, rms_w):
    cosT, sinT, tri, eye, eye128, perm = _host_tables()
    xf = np.asarray(x, dtype=np.float32).reshape(BT, C)
    xT = np.ascontiguousarray(xf.T)
    w = np.asarray(w_qkv, dtype=np.float32)
    rw = np.asarray(rms_w, dtype=np.float32)
    in_maps = []
    for i in range(NCORES):
        rows = slice(i * CSH, (i + 1) * CSH)
        # weights are scaled x64 (fp8 e4m3 normal range for V; harmless in
        # bf16 for Q/K); the 1/64 is folded back via the rsqrt row, and
        # 1/sqrt(D) via the exp input scale on-device
        wq = w[0 * C:1 * C][rows] * rw[None, :]
        wk = w[1 * C:2 * C][rows] * rw[None, :]
        wv = w[2 * C:3 * C][rows] * rw[None, :] * 64.0
        wT = np.concatenate([wq, wk], axis=0).T * 64.0   # [C, 2*CSH]
        # V -> [p, (P, i, m)] with c = P*256 + i*128 + p
        w8v = wv.reshape(CSH, CT // 2, 2, 128).transpose(3, 1, 2, 0)
        w8v = np.ascontiguousarray(w8v.reshape(128, -1))
        in_maps.append({
            "xT": xT.astype(BF16), "x8": xT.astype(FP8),
            "wT": np.ascontiguousarray(wT).astype(BF16),
            "w8v": w8v.astype(FP8),
            "wTv": np.ascontiguousarray(wv.T).astype(BF16),
            "cosT": cosT, "sinT": sinT, "tri": tri, "eye": eye,
            "eye128": eye128, "perm": perm,
        })
    return in_maps


def kernel(x, attention_mask, w_qkv, b_qkv, w_o, b_o, rms_w):
    from concourse.bass_utils import run_bass_kernel_spmd

    if "nc" not in _cache:
        _cache["nc"] = _build()
    nc = _cache["nc"]

    in_maps = _prep_inputs(x, w_qkv, rms_w)
    wo = np.asarray(w_o, dtype=np.float32)
    for i in range(NCORES):
        cols = slice(i * CSH, (i + 1) * CSH)
        in_maps[i]["woT"] = np.ascontiguousarray(wo[:, cols].T).astype(BF16)

    res = run_bass_kernel_spmd(nc, in_maps, core_ids=list(range(NCORES)))

    acc = np.zeros((BT, C), dtype=np.float32)
    for i in range(NCORES):
        acc += res.results[i]["out"].astype(np.float32)
    acc += np.asarray(b_o, dtype=np.float32)[None, :]
    return acc.reshape(B, T, C)
